# revision 42
# baseline (speedup 1.0000x reference)
"""GCN message-passing Bass kernel for TRN2 (8 cores).

Math: delta = segment_sum(w_e * x[src_e]) @ W^T  (linearity: transform after
aggregate).

Sharding: 4 source-quarters x 2 target-halves = 8 cores. Each core handles
edges whose source lies in its 25k-row quarter of x and whose target lies in
its 50k half; it produces a partial (transposed) delta for its half. The host
sums the 4 quarter-partials per half (the all-reduce/unshard step) -- valid
because delta = (sum_q agg_q) @ W^T is linear.

Per core, targets are degree-sorted (per-core degrees) into 128-target
blocks; block j has d_j padded edge slots. Slots are fetched by 1024-index
ANT dma_gathers (int16 indices into the 25k-row quarter -- this is why the
source dimension is quartered; 1024 = SWDGE ring capacity), amortizing the
~1us SWDGE descriptor-generation cost that dominated the per-slot
indirect-DMA baseline. DVE multiplies each gather slice by per-slot weights
and does a strided per-block reduce; PE transposes agg and applies W^T in
4-block groups; the Activation engine does the PSUM->SBUF copies; outputs
are written as contiguous [64, cols] bf16 (no indirect scatter) and
un-permuted on host.
"""

import numpy as np
from contextlib import ExitStack
from ml_dtypes import bfloat16

import concourse.bass as bass
import concourse.bacc as bacc
import concourse.mybir as mybir
import concourse.tile as tile
from concourse import library_config
from concourse.bass_utils import run_bass_kernel_spmd

P = 128
D = 64
NQ = 4          # source quarters
NH = 2          # target halves
QS = 25000      # sources per quarter
HS = 50000      # targets per half
NT_PAD = 50048  # targets padded to block multiple
NBLK = NT_PAD // P  # 391
N_CORES = 8
SC_COLS = 64       # columns per superchunk (mult granularity / msg tile size)
GA_COLS = 8        # columns per dma_gather (1024 idx = SWDGE ring capacity)
N_QUEUES = 4
GRP = 4

F32 = mybir.dt.float32
BF16 = mybir.dt.bfloat16
I16 = mybir.dt.int16


def preprocess(source, target, edge_weights):
    """Shared block schedule + per-core gather/weight arrays.

    Returns dict with d_sched (shared per-block slot counts, skipped leading
    zero blocks removed), nskip, S, chunks, and per-core idx/wgt arrays and
    rank (sorted position of each local target).
    """
    source = np.asarray(source).astype(np.int64)
    target = np.asarray(target).astype(np.int64)
    w_all = np.asarray(edge_weights).astype(np.float32)

    cores = []
    for q in range(NQ):
        for h in range(NH):
            m = ((source >= q * QS) & (source < (q + 1) * QS)
                 & (target >= h * HS) & (target < (h + 1) * HS))
            sl = (source[m] - q * QS).astype(np.int64)
            tl = (target[m] - h * HS).astype(np.int64)
            w = w_all[m]
            deg = np.bincount(tl, minlength=NT_PAD)
            perm = np.argsort(deg, kind="stable")
            rank = np.empty(NT_PAD, dtype=np.int64)
            rank[perm] = np.arange(NT_PAD)
            dmax = deg[perm].reshape(NBLK, P).max(axis=1)
            cores.append(dict(sl=sl, tl=tl, w=w, deg=deg, rank=rank, dmax=dmax))

    dmax_sh = np.max([c["dmax"] for c in cores], axis=0)
    nskip = int(np.argmax(dmax_sh > 0))  # leading all-zero blocks
    d_blocks = dmax_sh[nskip:].astype(np.int64)
    K = len(d_blocks)
    # processing order: rotate the smallest (first) block to the end so the
    # pipeline tail (last superchunk's DVE chain) is minimal
    proc = np.concatenate([np.arange(1, K), [0]])
    posn = np.empty(K, dtype=np.int64)
    posn[proc] = np.arange(K)
    d_sched = d_blocks[proc]
    offs_p = np.concatenate([[0], np.cumsum(d_sched)])
    S = int(offs_p[-1])

    # superchunks: consecutive blocks with total cols <= cap; the cap tapers
    # near the end (and the rotated-in small block stays alone) so the
    # pipeline tail -- DVE work trailing the last gather -- is minimal
    total = int(np.sum(d_sched[:K - 1]))
    chunks = []  # (first_block_rel, n_blocks, n_cols)
    j, done = 0, 0
    while j < K - 1:
        left = total - done
        cap = SC_COLS if left > 2 * SC_COLS else max(GA_COLS, left // 4)
        j0, cols = j, 0
        while j < K - 1 and cols + d_sched[j] <= cap:
            cols += int(d_sched[j])
            j += 1
        if j == j0:  # single block exceeds the tapered cap
            cols = int(d_sched[j])
            j += 1
        chunks.append((j0, j - j0, cols))
        done += cols
    chunks.append((K - 1, 1, int(d_sched[K - 1])))

    per_core = []
    for c in cores:
        deg, rank = c["deg"], c["rank"]
        # CSR by local target
        order = np.argsort(c["tl"], kind="stable")
        sl_s, w_s, tl_s = c["sl"][order], c["w"][order], c["tl"][order]
        starts = np.concatenate([[0], np.cumsum(deg)])
        slot = np.arange(len(tl_s)) - starts[tl_s]
        r = rank[tl_s]
        jabs = r // P
        p = r % P
        col = offs_p[posn[jabs - nskip]] + slot  # global schedule column
        pos = col * P + p

        idx_list = np.zeros(S * P, dtype=np.int16)
        idx_list[pos] = sl_s.astype(np.int16)
        # index i lives at [16*g + i%16, i//16] for all 8 gpsimd stripes
        idx_arr = np.tile(idx_list.reshape(S * 8, 16).T, (8, 1))
        wgt = np.zeros((P, S), dtype=bfloat16)
        wgt[p, col] = w_s.astype(bfloat16)
        per_core.append(dict(idx=np.ascontiguousarray(idx_arr), wgt=wgt,
                             rank=rank))

    return dict(d_sched=[int(d) for d in d_sched], nskip=nskip, S=S,
                chunks=chunks, per_core=per_core, posn=posn)


def build_nc(d_sched, chunks, S, bufs=2):
    nc = bacc.Bacc("TRN2", target_bir_lowering=False, debug=False,
                   num_swdge_queues=N_QUEUES)
    ncols = len(d_sched) * P
    maxc = max(SC_COLS, max(c[2] for c in chunks))  # tile capacity guard
    xq_t = nc.dram_tensor("xq", [QS, D], F32, kind="ExternalInput")
    idx_t = nc.dram_tensor("idx", [P, S * 8], I16, kind="ExternalInput")
    wgt_t = nc.dram_tensor("wgt", [P, S], BF16, kind="ExternalInput")
    wt_t = nc.dram_tensor("wT", [D, D], BF16, kind="ExternalInput")
    eye_t = nc.dram_tensor("eye", [P, P], F32, kind="ExternalInput")
    out_t = nc.dram_tensor("out", [D, ncols], BF16, kind="ExternalOutput")

    with tile.TileContext(nc) as tc, ExitStack() as ctx:
        nc.gpsimd.load_library(library_config.mlp)
        const = ctx.enter_context(tc.tile_pool(name="const", bufs=1))
        gpool = ctx.enter_context(tc.tile_pool(name="gather", bufs=bufs))
        mpool = ctx.enter_context(tc.tile_pool(name="msg", bufs=bufs))
        apool = ctx.enter_context(tc.tile_pool(name="agg", bufs=2 * GRP))
        tpool = ctx.enter_context(tc.tile_pool(name="aggT", bufs=3))
        opool = ctx.enter_context(tc.tile_pool(name="osb", bufs=3))
        psumT = ctx.enter_context(tc.tile_pool(name="psumT", bufs=3, space="PSUM"))
        psumM = ctx.enter_context(tc.tile_pool(name="psumM", bufs=3, space="PSUM"))

        idx_sb = const.tile([P, S * 8], I16)
        bounds = [0, GA_COLS] + list(range(128, S, 128)) + [S]
        for i, e in zip(bounds, bounds[1:]):
            if e > i:
                nc.sync.dma_start(out=idx_sb[:, i * 8:e * 8],
                                  in_=idx_t.ap()[:, i * 8:e * 8])
        wgt_sb = const.tile([P, S], BF16)
        nc.sync.dma_start(out=wgt_sb[:], in_=wgt_t.ap())
        wt_sb = const.tile([D, D], BF16)
        nc.sync.dma_start(out=wt_sb[:], in_=wt_t.ap())
        eye_sb = const.tile([P, P], F32)
        nc.sync.dma_start(out=eye_sb[:], in_=eye_t.ap())

        # Prime engines on the upfront loads so later instructions carry at
        # most one sync wait each.
        prime = const.tile([P, 1], BF16)
        nc.vector.tensor_copy(out=prime[:], in_=wgt_sb[:, :1])
        prime_a = const.tile([P, 1], BF16)
        nc.scalar.copy(out=prime_a[:], in_=wgt_sb[:, :1])
        prime_ps = psumT.tile([D, GRP * P], F32, tag="tp")
        nc.tensor.transpose(out=prime_ps[:, :P], in_=eye_sb[:, :D],
                            identity=eye_sb[:])

        group = []          # agg tiles pending transpose+transform
        gcol = 0            # output column of first block in group
        osb_cur = [None, 0, 0]  # tile, fill cols, start col

        def flush_osb():
            tile_, fill, start = osb_cur
            if tile_ is None or fill == 0:
                return
            nc.sync.dma_start(out=out_t.ap()[:, start:start + fill],
                              in_=tile_[:, :fill])
            osb_cur[0], osb_cur[1] = None, 0

        def flush_group():
            nonlocal group, gcol
            n = len(group)
            if n == 0:
                return
            w = n * P
            psT = psumT.tile([D, GRP * P], F32, tag="tp")
            for i, a in enumerate(group):
                nc.tensor.transpose(out=psT[:, i * P:(i + 1) * P], in_=a[:],
                                    identity=eye_sb[:])
            aggT = tpool.tile([D, GRP * P], BF16, tag="aT")
            nc.scalar.copy(out=aggT[:, :w], in_=psT[:, :w])
            ps2 = psumM.tile([D, GRP * P], F32, tag="mm")
            nc.tensor.matmul(out=ps2[:, :w], lhsT=wt_sb[:], rhs=aggT[:, :w],
                             start=True, stop=True)
            if osb_cur[0] is None:
                osb_cur[0] = opool.tile([D, 4 * GRP * P], BF16, tag="o",
                                        name="osb")
                osb_cur[1] = 0
                osb_cur[2] = gcol
            f = osb_cur[1]
            nc.scalar.copy(out=osb_cur[0][:, f:f + w], in_=ps2[:, :w])
            osb_cur[1] = f + w
            if osb_cur[1] == 4 * GRP * P:
                flush_osb()
            group = []
            gcol += w

        col = 0
        qn = 0
        for ci, (j0, nblks, ccols) in enumerate(chunks):
            if ci == len(chunks) - 1:
                flush_group()  # keep the final group down to the last block
            g = gpool.tile([P, maxc * D], F32, tag="g")
            msg = mpool.tile([P, maxc * D], BF16, tag="m")
            for c0 in range(0, ccols, GA_COLS):
                cc = min(GA_COLS, ccols - c0)
                nc.gpsimd.dma_gather(
                    out_ap=g[:, c0 * D:(c0 + cc) * D].rearrange(
                        "p (c o) -> p c o", o=D),
                    in_ap=xq_t.ap(),
                    idxs_ap=idx_sb[:, (col + c0) * 8:(col + c0 + cc) * 8],
                    num_idxs=cc * P,
                    num_idxs_reg=cc * P,
                    elem_size=D,
                    queue_num=qn,
                )
                qn = (qn + 1) % N_QUEUES
                nc.vector.tensor_tensor(
                    out=msg[:, c0 * D:(c0 + cc) * D].rearrange(
                        "p (c o) -> p c o", o=D),
                    in0=g[:, c0 * D:(c0 + cc) * D].rearrange(
                        "p (c o) -> p c o", o=D),
                    in1=wgt_sb[:, col + c0:col + c0 + cc].to_broadcast(
                        [P, cc, D]),
                    op=mybir.AluOpType.mult,
                )
            o = 0
            for jj in range(nblks):
                dj = d_sched[j0 + jj]
                agg = apool.tile([P, D], F32, tag="agg")
                nc.vector.tensor_reduce(
                    out=agg[:],
                    in_=msg[:, o * D:(o + dj) * D].rearrange(
                        "p (d o) -> p o d", o=D),
                    axis=mybir.AxisListType.X,
                    op=mybir.AluOpType.add,
                )
                group.append(agg)
                if len(group) == GRP:
                    flush_group()
                o += dj
            col += ccols
        flush_group()
        flush_osb()
    nc.compile()
    return nc


def run_gcn(x, W, edge_weights, source, target, num_nodes, trace=False, bufs=6):
    """Full-input host entry: preprocess, build, run on 8 cores, assemble."""
    assert int(num_nodes) == NQ * QS
    x = np.ascontiguousarray(np.asarray(x), dtype=np.float32)
    pp = preprocess(source, target, edge_weights)
    nc = build_nc(pp["d_sched"], pp["chunks"], pp["S"], bufs=bufs)

    wt_np = np.ascontiguousarray(np.asarray(W).T).astype(bfloat16)
    eye_np = np.eye(P, dtype=np.float32)
    in_maps = []
    for q in range(NQ):
        for h in range(NH):
            pc = pp["per_core"][q * NH + h]
            in_maps.append({
                "xq": x[q * QS:(q + 1) * QS],
                "idx": pc["idx"], "wgt": pc["wgt"],
                "wT": wt_np, "eye": eye_np,
            })
    res = run_bass_kernel_spmd(nc, in_maps, core_ids=list(range(N_CORES)),
                               trace=trace)

    nskip, posn = pp["nskip"], pp["posn"]
    # map sorted-rank -> device output column (-1 for skipped zero blocks)
    r = np.arange(NT_PAD)
    jrel = r // P - nskip
    colmap = np.where(jrel >= 0, posn[np.maximum(jrel, 0)] * P + r % P, -1)
    delta = np.zeros((NQ * QS, D), dtype=np.float32)
    tids = np.arange(HS)
    for q in range(NQ):
        for h in range(NH):
            k = q * NH + h
            out = np.asarray(res.results[k]["out"]).astype(np.float32)
            out_ext = np.concatenate(
                [out, np.zeros((D, 1), dtype=np.float32)], axis=1)
            rank = pp["per_core"][k]["rank"]
            delta[h * HS:(h + 1) * HS] += out_ext[:, colmap[rank[tids]]].T
    return delta, res


def kernel(**inputs) -> np.ndarray:
    """Harness entry: full unsharded inputs -> full (num_nodes, 64) output."""
    out, _ = run_gcn(
        np.asarray(inputs["x"]),
        np.asarray(inputs["W"]),
        np.asarray(inputs["edge_weights"]),
        np.asarray(inputs["source"]),
        np.asarray(inputs["target"]),
        int(inputs["num_nodes"]),
        trace=False,
    )
    return out
